# revision 24
# baseline (speedup 1.0000x reference)
"""Multi-head attention (RoPE + causal SDPA) on 8 Trainium2 NeuronCores.

Tensor-parallel over heads (2 heads/core) for QKV+attention; per-(batch,
half) AllToAll reshards head-split -> row-split; each core out-projects
its row slice with the full wo.

v3 notes:
  - Packed DRAM layouts; big-descriptor DMAs only.
  - V computed in natural (kpos, hd) layout (x-chunk stationary).
  - Two heads' K=64 score matmuls issued back-to-back -> concurrent PE
    row-tiles (0,0)/(64,0) (verified in trace: 2nd of pair ~3ns).
  - Scores for a kb-pair land in one 2-bank PSUM tile [128,1024]; one
    exp ACTIVATE covers both blocks. Causal masks as ONE wide [128,1024]
    tensor_tensor per (qtile, head, diag-pair), split DVE/GpSimd.
  - Elementwise rebalanced: GpSimd only gets full-128-partition
    SBUF-only multiplies (it is ~2.6x slower than DVE and terrible on
    <128-partition ops); DVE keeps PSUM readers + small copies.
  - phaseA(1) interleaved into phaseB(0) at q-tile granularity so PE has
    dense work while Scalar crunches exps; collectives start early.
  - Output stored bf16 (host casts to f32).
"""
import sys, os
if '/opt/trn_rl_repo' not in sys.path:
    sys.path.insert(0, '/opt/trn_rl_repo')
os.environ.setdefault('MYCRO_LOCAL_CACHE', '1')

from contextlib import ExitStack

import numpy as np
import ml_dtypes

import concourse.bass as bass
import concourse.tile as tile
from concourse import bacc, mybir
from concourse.bass_utils import run_bass_kernel_spmd

BF16 = ml_dtypes.bfloat16
NC = 8           # cores
B = 2            # batch
D = 1024         # model dim
H = 16           # heads
HD = 64          # head dim
HPC = H // NC    # heads per core = 2
DPC = HPC * HD   # head dims per core = 128
ROPE_BASE = 10000.0
QT = 512         # q tile width
KB = 128         # k block (partition axis of score matmuls)
DCH = D // 128   # contraction chunks (= 8)

F32 = mybir.dt.float32
BF = mybir.dt.bfloat16
MULT = mybir.AluOpType.mult
ADD = mybir.AluOpType.add

# csm (cos/sin/masks) column offsets
COS0, SIN0, MSK0 = 0, 2048, 4096
CSM_COLS = MSK0 + 2 * 2 * QT     # cos|sin|wmask01|wmask23 = 6144


def build_nc(S):
    RH = S // NC           # rows per core per batch = 256
    NST = S // QT          # 4 seq tiles per batch
    NVT = S // KB          # 16 V k-blocks per batch
    NHALF = 2
    RHH = RH // NHALF      # 128

    nc = bacc.Bacc(num_devices=NC)

    wqkv = nc.declare_dram_parameter("wqkv", [128, 3 * D], BF, isOutput=False)
    csm = nc.declare_dram_parameter("csm", [128, CSM_COLS], BF, isOutput=False)
    woP = nc.declare_dram_parameter("woP", [128, DCH * D], BF, isOutput=False)
    xP = nc.declare_dram_parameter("xP", [128, B * NST, DCH * QT], BF,
                                   isOutput=False)
    out = nc.declare_dram_parameter("out", [128, B * DCH * RH], BF,
                                    isOutput=True)

    a2a_in = {0: nc.dram_tensor("a2a_in0", [NC, 128, RH], BF)}
    a2a_out = {0: nc.dram_tensor("a2a_out0", [NC, 128, RH], BF)}
    for h in range(NHALF):
        a2a_in[(1, h)] = nc.dram_tensor(f"a2a_in1_{h}", [NC, 128, RHH], BF)
        a2a_out[(1, h)] = nc.dram_tensor(f"a2a_out1_{h}", [NC, 128, RHH], BF)
    dsync_in = nc.dram_tensor("dsync_in", [NC, 1, 16], BF)
    dsync_out = nc.dram_tensor("dsync_out", [NC, 1, 16], BF)

    ctx = ExitStack()
    with ctx:
        tc = ctx.enter_context(tile.TileContext(nc))

        consts = ctx.enter_context(tc.tile_pool(name="consts", bufs=1))
        xpool = ctx.enter_context(tc.tile_pool(name="x", bufs=2 * NST))
        pQt = ctx.enter_context(tc.tile_pool(name="qt", bufs=2))
        pKt = ctx.enter_context(tc.tile_pool(name="kt", bufs=2))
        pV = ctx.enter_context(tc.tile_pool(name="v", bufs=2))
        pO = ctx.enter_context(tc.tile_pool(name="oall", bufs=2))
        ptmp = ctx.enter_context(tc.tile_pool(name="tmp", bufs=4))
        ppt = ctx.enter_context(tc.tile_pool(name="pt", bufs=2))
        pnorm = ctx.enter_context(tc.tile_pool(name="norm", bufs=3))
        pog = ctx.enter_context(tc.tile_pool(name="og", bufs=2))
        posb = ctx.enter_context(tc.tile_pool(name="osb", bufs=2))

        # PSUM: 8 banks = ppA(2) + ps(4: both heads x kb-pair) + po0/po1(2)
        ppA = ctx.enter_context(tc.tile_pool(name="ppA", bufs=2, space="PSUM"))
        pps = ctx.enter_context(tc.tile_pool(name="pps", bufs=1, space="PSUM"))
        ppo = ctx.enter_context(tc.tile_pool(name="ppo", bufs=1, space="PSUM"))

        # launch-skew absorber: cores rendezvous here while input DMAs
        # stream in (saves the first real collective from eating the skew)
        nc.gpsimd.collective_compute(
            "AllToAll", mybir.AluOpType.bypass,
            replica_groups=[list(range(NC))],
            ins=[dsync_in[:].opt()], outs=[dsync_out[:].opt()])

        # ---- constants ----
        # wqkv split q|k|v so the first LDWEIGHTS waits only on the q part
        wqkv_sb = consts.tile([128, 3 * D], BF, tag="wqkv")
        nc.sync.dma_start(out=wqkv_sb[:, 0:D], in_=wqkv[:, 0:D])

        def w_sl(which, d):
            base = {'q': 0, 'k': D, 'v': 2 * D}[which]
            return wqkv_sb[:, base + d * 128:base + (d + 1) * 128]

        # x seq-blocks: batch0 st0, then csm, then rest
        xts = {}

        def load_x(b, st, chunks=1, eng=None):
            t = xpool.tile([128, DCH * QT], BF, tag="x", name="xt")
            cw = (DCH // chunks) * QT
            for ci in range(chunks):
                (eng or nc.sync).dma_start(out=t[:, ci * cw:(ci + 1) * cw],
                                           in_=xP[:, b * NST + st,
                                                  ci * cw:(ci + 1) * cw])
            xts[(b, st)] = t

        load_x(0, 0, chunks=8)
        nc.sync.dma_start(out=wqkv_sb[:, D:3 * D], in_=wqkv[:, D:3 * D])
        load_x(0, 1)
        csm_sb = consts.tile([128, CSM_COLS], BF, tag="csm")
        nc.sync.dma_start(out=csm_sb[:, 0:MSK0], in_=csm[:, 0:MSK0])
        cos_sb = csm_sb[:, COS0:COS0 + S]
        sin_sb = csm_sb[:, SIN0:SIN0 + S]

        def wmask(half):   # [128, 1024] wide causal mask for diag pair
            return csm_sb[:, MSK0 + half * 2 * QT:MSK0 + (half + 1) * 2 * QT]

        load_x(0, 2)
        nc.sync.dma_start(out=csm_sb[:, MSK0:CSM_COLS],
                          in_=csm[:, MSK0:CSM_COLS])
        load_x(0, 3)
        ones_sb = consts.tile([1, HD], BF, tag="ones")
        nc.gpsimd.memset(ones_sb[:], 1.0)
        for st in range(NST):
            load_x(1, st)
        wo_sb = consts.tile([128, DCH * D], BF, tag="wo")
        nc.sync.dma_start(out=wo_sb[:], in_=woP[:, :])

        qt_t, kt_t, v_t, o_t = {}, {}, {}, {}
        for _b in range(B):
            _vt = pV.tile([128, NVT, 130], BF, tag="vt", name="vt")
            nc.gpsimd.memset(_vt[:, :, 64:65], 1.0)
            nc.gpsimd.memset(_vt[:, :, 129:130], 1.0)
            v_t[_b] = _vt

        # ---------------- phase A: st-major Q/K/V interleave ----------------
        def proj_rope_st(b, which, dest, st):
            xt = xts[(b, st)]
            ps = ppA.tile([128, QT], F32, tag="pA", name="psproj")
            for d in range(DCH):
                nc.tensor.matmul(
                    ps[:], w_sl(which, d), xt[:, d * QT:(d + 1) * QT],
                    start=(d == 0), stop=(d == DCH - 1))
            c0 = st * QT
            raw = ptmp.tile([128, QT], BF, tag="raw", name="raw")
            if b == 0:
                nc.scalar.activation(raw[:], ps[:],
                                     mybir.ActivationFunctionType.Copy)
            else:
                nc.vector.tensor_copy(raw[:], ps[:])
            tcos = ptmp.tile([128, QT], BF, tag="tcos", name="tcos")
            nc.vector.tensor_tensor(
                tcos[:], raw[:], cos_sb[:, c0:c0 + QT], MULT)
            trot = ptmp.tile([128, QT], BF, tag="trot", name="trot")
            for g in range(4):
                o0 = g * 32
                i0 = o0 + 32 if g % 2 == 0 else o0 - 32
                nc.vector.tensor_copy(trot[o0:o0 + 32, :],
                                      raw[i0:i0 + 32, :])
            nc.vector.tensor_tensor(
                trot[:], trot[:], sin_sb[:, c0:c0 + QT], MULT)
            nc.vector.tensor_tensor(
                dest[:, c0:c0 + QT], tcos[:], trot[:], ADD)

        def proj_v_st(b, vt, st):
            xt = xts[(b, st)]
            psv = ppA.tile([128, 4 * 128], F32, tag="pA", name="psv")
            for i in range(4):
                kcols = i * 128
                for d in range(DCH):
                    nc.tensor.matmul(
                        psv[:, kcols:kcols + 128],
                        xt[:, d * QT + kcols:d * QT + kcols + 128],
                        w_sl('v', d),
                        start=(d == 0), stop=(d == DCH - 1),
                        skip_group_check=True)
            pv3 = psv[:].rearrange("p (k c) -> p k c", c=128)
            vt3 = vt[:, st * 4:(st + 1) * 4, :]
            nc.vector.tensor_copy(vt3[:, :, 0:64], pv3[:, :, 0:64])
            nc.vector.tensor_copy(vt3[:, :, 65:129], pv3[:, :, 64:128])

        def phaseA(b, sts=None, parts='qkv'):
            if 'q' in parts and b not in qt_t:
                qt_t[b] = pQt.tile([128, S], BF, tag="qt", name="qtt")
            if 'k' in parts and b not in kt_t:
                kt_t[b] = pKt.tile([128, S], BF, tag="kt", name="ktt")
            for st in (sts if sts is not None else range(NST)):
                if 'q' in parts:
                    proj_rope_st(b, 'q', qt_t[b], st)
                if 'k' in parts:
                    proj_rope_st(b, 'k', kt_t[b], st)
                if 'v' in parts:
                    proj_v_st(b, v_t[b], st)

        # ---------------- phase B: attention ----------------
        def phaseB(b, qts):
            if b not in o_t:
                o_t[b] = pO.tile([128, S], BF, tag="ob", name="ob")
            ob = o_t[b]
            qtt, ktt, vt = qt_t[b], kt_t[b], v_t[b]
            for qt_i in qts:
                q0 = qt_i * QT
                nkb = (q0 + QT) // KB
                po = {}
                for h in range(HPC):
                    po[h] = ppo.tile([128, QT], F32, tag=f"po{h}",
                                     name=f"po{h}")

                def pv_quad(kp, pt_h):
                    for i in range(2):
                        kb = 2 * kp + i
                        for h in range(HPC):
                            nc.tensor.matmul(
                                po[h][0:65, :], vt[:, kb, 65 * h:65 * h + 65],
                                pt_h[h][:, i * QT:(i + 1) * QT],
                                start=(kb == 0), stop=(kb == nkb - 1),
                                skip_group_check=True)

                prev = None   # (kp, pt_h) held back one iteration
                for kp in range(nkb // 2):
                    kb0 = 2 * kp
                    # per-head 2-bank tiles: exp ping-pongs between heads
                    ps_h = {h: pps.tile([128, 2 * QT], F32, tag=f"ps{h}",
                                        name=f"ps{h}") for h in range(HPC)}
                    for i in range(2):
                        k0 = (kb0 + i) * KB
                        for h in range(HPC):
                            p0 = h * HD
                            nc.tensor.matmul(
                                ps_h[h][:, i * QT:(i + 1) * QT],
                                ktt[p0:p0 + HD, k0:k0 + KB],
                                qtt[p0:p0 + HD, q0:q0 + QT],
                                start=True, stop=True,
                                skip_group_check=True)
                    if prev is not None:
                        pv_quad(*prev)
                    pt_h = {}
                    for h in range(HPC):
                        pt = ppt.tile([128, 2 * QT], BF, tag=f"pt{h}",
                                      name=f"pt{h}")
                        nc.scalar.activation(
                            pt[:], ps_h[h][:], mybir.ActivationFunctionType.Exp,
                            scale=float(HD) ** -0.5)
                        if kp >= 2 * qt_i:
                            half = kp - 2 * qt_i
                            nc.vector.tensor_tensor(pt[:], pt[:], wmask(half),
                                                    MULT)
                        pt_h[h] = pt
                    prev = (kp, pt_h)
                pv_quad(*prev)
                # normalize: O/l ; l in po row HD; r broadcast via PE
                bcps = ppA.tile([128, QT], F32, tag="pA", name="bcps")
                for h in range(HPC):
                    lsb = pnorm.tile([1, QT], F32, tag=f"lsb{h}", name="lsb")
                    nc.vector.tensor_copy(lsb[:], po[h][HD:HD + 1, :])
                    recip32 = pnorm.tile([1, QT], F32, tag=f"recip32{h}",
                                         name="recip32")
                    nc.vector.reciprocal_approx_fast(recip32[:], lsb[:])
                    recipb = pnorm.tile([1, QT], BF, tag=f"recipb{h}",
                                        name="recipb")
                    nc.vector.tensor_copy(recipb[:], recip32[:])
                    nc.tensor.matmul(
                        bcps[h * HD:(h + 1) * HD, :], ones_sb[:, 0:HD],
                        recipb[:], start=True, stop=True,
                        skip_group_check=True)
                bc = pnorm.tile([2 * HD, QT], BF, tag="bc", name="bc")
                nc.vector.tensor_copy(bc[:], bcps[:])
                for h in range(HPC):
                    nc.vector.tensor_tensor(
                        ob[h * HD:(h + 1) * HD, q0:q0 + QT],
                        po[h][0:HD, :], bc[h * HD:(h + 1) * HD, :], MULT)

        # ---------------- phase C: reshard (collective) ----------------
        def phaseC0():
            for j in range(NC):
                nc.sync.dma_start(
                    out=a2a_in[0][j, :, :],
                    in_=o_t[0][:, j * RH:(j + 1) * RH])
            nc.gpsimd.collective_compute(
                "AllToAll", mybir.AluOpType.bypass,
                replica_groups=[list(range(NC))],
                ins=[a2a_in[0][:].opt()], outs=[a2a_out[0][:].opt()])

        def phaseC1(h):
            base = h * (S // NHALF)
            for j in range(NC):
                nc.sync.dma_start(
                    out=a2a_in[(1, h)][j, :, :],
                    in_=o_t[1][:, base + j * RHH:base + (j + 1) * RHH])
            nc.gpsimd.collective_compute(
                "AllToAll", mybir.AluOpType.bypass,
                replica_groups=[list(range(NC))],
                ins=[a2a_in[(1, h)][:].opt()],
                outs=[a2a_out[(1, h)][:].opt()])

        # ---------------- phase D: out-projection ----------------
        def phaseD0():
            og = pog.tile([128, NC * RH], BF, tag="og", name="og")
            for d in range(NC):
                nc.sync.dma_start(out=og[:, d * RH:(d + 1) * RH],
                                  in_=a2a_out[0][d, :, :])
            osb = posb.tile([128, DCH * RH], BF, tag="osb", name="osb")
            for e in range(DCH):
                pso = ppA.tile([128, RH], F32, tag="pA", name="pso")
                for d in range(NC):
                    nc.tensor.matmul(
                        pso[:], wo_sb[:, d * D + e * 128:d * D + (e + 1) * 128],
                        og[:, d * RH:(d + 1) * RH],
                        start=(d == 0), stop=(d == NC - 1))
                nc.vector.tensor_copy(osb[:, e * RH:(e + 1) * RH], pso[:])
            nc.sync.dma_start(out=out[:, 0:DCH * RH], in_=osb[:])

        def phaseD1(h):
            og = pog.tile([128, NC * RHH], BF, tag="og", name="og1")
            for d in range(NC):
                nc.sync.dma_start(out=og[:, d * RHH:(d + 1) * RHH],
                                  in_=a2a_out[(1, h)][d, :, :])
            osb = posb.tile([128, DCH * RHH], BF, tag="osb", name="osb1")
            for e in range(DCH):
                pso = ppA.tile([128, RHH], F32, tag="pA", name="pso1")
                for d in range(NC):
                    nc.tensor.matmul(
                        pso[:], wo_sb[:, d * D + e * 128:d * D + (e + 1) * 128],
                        og[:, d * RHH:(d + 1) * RHH],
                        start=(d == 0), stop=(d == NC - 1))
                nc.vector.tensor_copy(osb[:, e * RHH:(e + 1) * RHH], pso[:])
            out_v = out[:, DCH * RH:2 * DCH * RH].rearrange(
                "p (e r) -> p e r", e=DCH)[:, :, h * RHH:(h + 1) * RHH]
            nc.sync.dma_start(
                out=out_v,
                in_=osb[:].rearrange("p (e r) -> p e r", e=DCH))

        # ---------------- schedule ----------------
        # CC queue pipelines C0 -> C1(0) -> C1(1) back-to-back; D0 fills
        # B(1,[2,3]) bubbles, D1(0) fills the C1(1) window, only D1(1)
        # remains after the last collective.
        phaseA(0)
        phaseB(0, [0, 1])
        phaseA(1, parts='q')
        phaseB(0, [2])
        phaseA(1, parts='k')
        phaseB(0, [3])
        phaseC0()
        phaseA(1, parts='v')
        phaseB(1, [2, 3])
        phaseC1(1)
        phaseB(1, [0, 1])
        phaseC1(0)
        phaseD0()
        phaseD1(1)
        phaseD1(0)

    nc.compile()
    return nc


_NC_CACHE = {}


def _get_nc(S):
    if S not in _NC_CACHE:
        _NC_CACHE[S] = build_nc(S)
    return _NC_CACHE[S]


def make_in_maps(x, wq, wk, wv, wo):
    b, S, d = x.shape
    NST = S // QT

    # xP[p, b*NST+st, d*QT+c] = x[b, st*QT+c, d*128+p]
    xP = np.ascontiguousarray(
        x.reshape(B, NST, QT, DCH, 128).transpose(4, 0, 1, 3, 2)
        .reshape(128, B * NST, DCH * QT)).astype(BF16)

    # woP[p, d*D+e] = wo[e, d*128+p]
    woP = np.ascontiguousarray(
        wo.T.reshape(DCH, 128, D).transpose(1, 0, 2).reshape(128, DCH * D)
    ).astype(BF16)

    # RoPE tables (transposed, both 64-dim head copies stacked)
    inv = (1.0 / ROPE_BASE ** (np.arange(0, HD, 2, dtype=np.float64) / HD))
    t = np.arange(S, dtype=np.float64)
    fr = np.outer(t, inv)                      # [S, 32]
    emb = np.concatenate([fr, fr], axis=1)     # [S, 64]
    cos_t = np.cos(emb).T                      # [64, S]
    sin_t = np.sin(emb).T
    sgn = np.where(np.arange(HD) < HD // 2, -1.0, 1.0)[:, None]
    cosT = np.concatenate([cos_t, cos_t], axis=0)          # [128, S]
    sinT = np.concatenate([sin_t * sgn, sin_t * sgn], axis=0)

    pp = np.arange(128)[:, None]
    qn = np.arange(QT)[None, :]
    # wide masks: [mask_0|mask_1] and [mask_2|mask_3]
    masks = [(qn >= j * KB + pp) for j in range(4)]
    wm01 = np.concatenate([masks[0], masks[1]], axis=1)
    wm23 = np.concatenate([masks[2], masks[3]], axis=1)

    csm = np.concatenate([cosT, sinT, wm01, wm23], axis=1).astype(BF16)
    assert csm.shape[1] == CSM_COLS

    def wpack(w, sl):
        return w[sl, :].T.reshape(DCH, 128, DPC).transpose(1, 0, 2).reshape(
            128, DCH * DPC)

    in_maps = []
    for c in range(NC):
        sl = slice(c * DPC, (c + 1) * DPC)
        wqkv = np.concatenate(
            [wpack(wq, sl), wpack(wk, sl), wpack(wv, sl)], axis=1).astype(BF16)
        in_maps.append({
            "wqkv": np.ascontiguousarray(wqkv),
            "csm": np.ascontiguousarray(csm),
            "woP": woP,
            "xP": xP,
        })
    return in_maps


def assemble(outs, S):
    """outs[c] [128, B*DCH*RH] -> full (B, S, D) f32.

    Batch 0: contiguous RH rows per core. Batch 1: two halves of RHH.
    """
    RH = S // NC
    RHH = RH // 2
    full = np.empty((B, S, D), dtype=np.float32)
    for c in range(NC):
        o = np.asarray(outs[c]).astype(np.float32).reshape(128, B, DCH, RH)
        blk = o.transpose(1, 3, 2, 0).reshape(B, RH, D)
        full[0, c * RH:(c + 1) * RH, :] = blk[0]
        for h in range(2):
            r0 = h * (S // 2) + c * RHH
            full[1, r0:r0 + RHH, :] = blk[1, h * RHH:(h + 1) * RHH]
    return full


def run(x, wq, wk, wv, wo, trace=False):
    b, S, d = x.shape
    nc = _get_nc(S)
    in_maps = make_in_maps(x, wq, wk, wv, wo)
    res = run_bass_kernel_spmd(nc, in_maps, core_ids=list(range(NC)),
                               trace=trace)
    full = assemble([res.results[c]["out"] for c in range(NC)], S)
    return full, res


def kernel(x, wq, wk, wv, wo):
    full, _ = run(np.asarray(x), np.asarray(wq), np.asarray(wk),
                  np.asarray(wv), np.asarray(wo))
    return full



# revision 28
# speedup vs baseline: 1.0357x; 1.0357x over previous
"""Multi-head attention (RoPE + causal SDPA) on 8 Trainium2 NeuronCores.

Tensor-parallel over heads (2 heads/core) for QKV+attention; per-(batch,
half) AllToAll reshards head-split -> row-split; each core out-projects
its row slice with the full wo.

v3 notes:
  - Packed DRAM layouts; big-descriptor DMAs only.
  - V computed in natural (kpos, hd) layout (x-chunk stationary).
  - Two heads' K=64 score matmuls issued back-to-back -> concurrent PE
    row-tiles (0,0)/(64,0) (verified in trace: 2nd of pair ~3ns).
  - Scores for a kb-pair land in one 2-bank PSUM tile [128,1024]; one
    exp ACTIVATE covers both blocks. Causal masks as ONE wide [128,1024]
    tensor_tensor per (qtile, head, diag-pair), split DVE/GpSimd.
  - Elementwise rebalanced: GpSimd only gets full-128-partition
    SBUF-only multiplies (it is ~2.6x slower than DVE and terrible on
    <128-partition ops); DVE keeps PSUM readers + small copies.
  - phaseA(1) interleaved into phaseB(0) at q-tile granularity so PE has
    dense work while Scalar crunches exps; collectives start early.
  - Output stored bf16 (host casts to f32).
"""
import sys, os
if '/opt/trn_rl_repo' not in sys.path:
    sys.path.insert(0, '/opt/trn_rl_repo')
os.environ.setdefault('MYCRO_LOCAL_CACHE', '1')

from contextlib import ExitStack
from itertools import chain

import numpy as np
import ml_dtypes

import concourse.bass as bass
import concourse.tile as tile
from concourse import bacc, mybir
from concourse.bass_utils import run_bass_kernel_spmd

BF16 = ml_dtypes.bfloat16
NC = 8           # cores
B = 2            # batch
D = 1024         # model dim
H = 16           # heads
HD = 64          # head dim
HPC = H // NC    # heads per core = 2
DPC = HPC * HD   # head dims per core = 128
ROPE_BASE = 10000.0
QT = 512         # q tile width
KB = 128         # k block (partition axis of score matmuls)
DCH = D // 128   # contraction chunks (= 8)

F32 = mybir.dt.float32
BF = mybir.dt.bfloat16
MULT = mybir.AluOpType.mult
ADD = mybir.AluOpType.add

# csm (cos/sin/masks) column offsets
COS0, SIN0, MSK0 = 0, 2048, 4096
CSM_COLS = MSK0 + 2 * 2 * QT     # cos|sin|wmask01|wmask23 = 6144


def build_nc(S):
    RH = S // NC           # rows per core per batch = 256
    NST = S // QT          # 4 seq tiles per batch
    NVT = S // KB          # 16 V k-blocks per batch
    NHALF = 2
    RHH = RH // NHALF      # 128

    nc = bacc.Bacc(num_devices=NC)

    wqkv = nc.declare_dram_parameter("wqkv", [128, 3 * D], BF, isOutput=False)
    csm = nc.declare_dram_parameter("csm", [128, CSM_COLS], BF, isOutput=False)
    woP = nc.declare_dram_parameter("woP", [128, DCH * D], BF, isOutput=False)
    xP = nc.declare_dram_parameter("xP", [128, B * NST, DCH * QT], BF,
                                   isOutput=False)
    out = nc.declare_dram_parameter("out", [128, B * DCH * RH], BF,
                                    isOutput=True)

    a2a_in = {0: nc.dram_tensor("a2a_in0", [NC, 128, RH], BF)}
    a2a_out = {0: nc.dram_tensor("a2a_out0", [NC, 128, RH], BF)}
    for h in range(NHALF):
        a2a_in[(1, h)] = nc.dram_tensor(f"a2a_in1_{h}", [NC, 128, RHH], BF)
        a2a_out[(1, h)] = nc.dram_tensor(f"a2a_out1_{h}", [NC, 128, RHH], BF)
    dsync_in = nc.dram_tensor("dsync_in", [NC, 1, 16], BF)
    dsync_out = nc.dram_tensor("dsync_out", [NC, 1, 16], BF)

    ctx = ExitStack()
    with ctx:
        tc = ctx.enter_context(tile.TileContext(nc))

        consts = ctx.enter_context(tc.tile_pool(name="consts", bufs=1))
        xpool = ctx.enter_context(tc.tile_pool(name="x", bufs=2 * NST))
        pQt = ctx.enter_context(tc.tile_pool(name="qt", bufs=2))
        pKt = ctx.enter_context(tc.tile_pool(name="kt", bufs=2))
        pV = ctx.enter_context(tc.tile_pool(name="v", bufs=2))
        pO = ctx.enter_context(tc.tile_pool(name="oall", bufs=2))
        ptmp = ctx.enter_context(tc.tile_pool(name="tmp", bufs=4))
        ppt = ctx.enter_context(tc.tile_pool(name="pt", bufs=2))
        pnorm = ctx.enter_context(tc.tile_pool(name="norm", bufs=3))
        pog = ctx.enter_context(tc.tile_pool(name="og", bufs=2))
        posb = ctx.enter_context(tc.tile_pool(name="osb", bufs=2))

        # PSUM: 8 banks = ppA(2) + ps(4: both heads x kb-pair) + po0/po1(2)
        ppA = ctx.enter_context(tc.tile_pool(name="ppA", bufs=2, space="PSUM"))
        pps = ctx.enter_context(tc.tile_pool(name="pps", bufs=1, space="PSUM"))
        ppo = ctx.enter_context(tc.tile_pool(name="ppo", bufs=1, space="PSUM"))

        # launch-skew absorber: cores rendezvous here while input DMAs
        # stream in (saves the first real collective from eating the skew)
        nc.gpsimd.collective_compute(
            "AllToAll", mybir.AluOpType.bypass,
            replica_groups=[list(range(NC))],
            ins=[dsync_in[:].opt()], outs=[dsync_out[:].opt()])

        # ---- constants ----
        # wqkv split q|k|v so the first LDWEIGHTS waits only on the q part
        wqkv_sb = consts.tile([128, 3 * D], BF, tag="wqkv")
        nc.sync.dma_start(out=wqkv_sb[:, 0:D], in_=wqkv[:, 0:D])

        def w_sl(which, d):
            base = {'q': 0, 'k': D, 'v': 2 * D}[which]
            return wqkv_sb[:, base + d * 128:base + (d + 1) * 128]

        # x seq-blocks: batch0 st0, then csm, then rest
        xts = {}

        def load_x(b, st, chunks=1, eng=None):
            t = xpool.tile([128, DCH * QT], BF, tag="x", name="xt")
            cw = (DCH // chunks) * QT
            for ci in range(chunks):
                (eng or nc.sync).dma_start(out=t[:, ci * cw:(ci + 1) * cw],
                                           in_=xP[:, b * NST + st,
                                                  ci * cw:(ci + 1) * cw])
            xts[(b, st)] = t

        load_x(0, 0, chunks=8)
        nc.sync.dma_start(out=wqkv_sb[:, D:3 * D], in_=wqkv[:, D:3 * D])
        load_x(0, 1)
        csm_sb = consts.tile([128, CSM_COLS], BF, tag="csm")
        nc.sync.dma_start(out=csm_sb[:, 0:MSK0], in_=csm[:, 0:MSK0])
        cos_sb = csm_sb[:, COS0:COS0 + S]
        sin_sb = csm_sb[:, SIN0:SIN0 + S]

        def wmask(half):   # [128, 1024] wide causal mask for diag pair
            return csm_sb[:, MSK0 + half * 2 * QT:MSK0 + (half + 1) * 2 * QT]

        load_x(0, 2)
        nc.sync.dma_start(out=csm_sb[:, MSK0:CSM_COLS],
                          in_=csm[:, MSK0:CSM_COLS])
        load_x(0, 3)
        ones_sb = consts.tile([1, HD], BF, tag="ones")
        nc.gpsimd.memset(ones_sb[:], 1.0)
        for st in range(NST):
            load_x(1, st)
        wo_sb = consts.tile([128, DCH * D], BF, tag="wo")
        nc.sync.dma_start(out=wo_sb[:], in_=woP[:, :])

        qt_t, kt_t, v_t, o_t = {}, {}, {}, {}
        for _b in range(B):
            _vt = pV.tile([128, NVT, 130], BF, tag="vt", name="vt")
            nc.gpsimd.memset(_vt[:, :, 64:65], 1.0)
            nc.gpsimd.memset(_vt[:, :, 129:130], 1.0)
            v_t[_b] = _vt

        # ---------------- phase A: st-major Q/K/V interleave ----------------
        # Emitted as small thunks (~4 matmuls each) so phaseB can pop one
        # per kp-pair, filling the PE bubble while Scalar crunches exps.
        def proj_rope_thunks(b, which, st):
            xt = xts[(b, st)]
            box = {}

            def t1():
                ps = ppA.tile([128, QT], F32, tag="pA", name="psproj")
                box['ps'] = ps
                for d in range(4):
                    nc.tensor.matmul(
                        ps[:], w_sl(which, d), xt[:, d * QT:(d + 1) * QT],
                        start=(d == 0), stop=False)

            def t2():
                ps = box['ps']
                dest = qt_t[b] if which == 'q' else kt_t[b]
                for d in range(4, DCH):
                    nc.tensor.matmul(
                        ps[:], w_sl(which, d), xt[:, d * QT:(d + 1) * QT],
                        start=False, stop=(d == DCH - 1))
                c0 = st * QT
                raw = ptmp.tile([128, QT], BF, tag="raw", name="raw")
                if b == 0:
                    nc.scalar.activation(raw[:], ps[:],
                                         mybir.ActivationFunctionType.Copy)
                else:
                    nc.vector.tensor_copy(raw[:], ps[:])
                tcos = ptmp.tile([128, QT], BF, tag="tcos", name="tcos")
                nc.vector.tensor_tensor(
                    tcos[:], raw[:], cos_sb[:, c0:c0 + QT], MULT)
                trot = ptmp.tile([128, QT], BF, tag="trot", name="trot")
                for g in range(4):
                    o0 = g * 32
                    i0 = o0 + 32 if g % 2 == 0 else o0 - 32
                    nc.vector.tensor_copy(trot[o0:o0 + 32, :],
                                          raw[i0:i0 + 32, :])
                nc.vector.tensor_tensor(
                    trot[:], trot[:], sin_sb[:, c0:c0 + QT], MULT)
                nc.vector.tensor_tensor(
                    dest[:, c0:c0 + QT], tcos[:], trot[:], ADD)

            return [t1, t2]

        def proj_v_thunks(b, st):
            xt = xts[(b, st)]
            box = {}

            def mk(i0):
                def t():
                    if 'ps' not in box:
                        box['ps'] = ppA.tile([128, 4 * 128], F32, tag="pA",
                                             name="psv")
                    psv = box['ps']
                    for i in (i0, i0 + 1):
                        kcols = i * 128
                        for d in range(DCH):
                            nc.tensor.matmul(
                                psv[:, kcols:kcols + 128],
                                xt[:, d * QT + kcols:d * QT + kcols + 128],
                                w_sl('v', d),
                                start=(d == 0), stop=(d == DCH - 1),
                                skip_group_check=True)
                    if i0 == 2:
                        vt = v_t[b]
                        pv3 = psv[:].rearrange("p (k c) -> p k c", c=128)
                        vt3 = vt[:, st * 4:(st + 1) * 4, :]
                        nc.vector.tensor_copy(vt3[:, :, 0:64], pv3[:, :, 0:64])
                        nc.vector.tensor_copy(vt3[:, :, 65:129],
                                              pv3[:, :, 64:128])
                return t

            return [mk(0), mk(2)]

        def a_thunks(b, parts):
            if 'q' in parts and b not in qt_t:
                qt_t[b] = pQt.tile([128, S], BF, tag="qt", name="qtt")
            if 'k' in parts and b not in kt_t:
                kt_t[b] = pKt.tile([128, S], BF, tag="kt", name="ktt")
            th = []
            for st in range(NST):
                for p in parts:
                    if p == 'v':
                        th += proj_v_thunks(b, st)
                    else:
                        th += proj_rope_thunks(b, p, st)
            return th

        def phaseA(b, parts='qkv'):
            for t in a_thunks(b, parts):
                t()

        def drain(it):
            for t in it:
                t()

        # ---------------- phase B: attention ----------------
        def phaseB(b, qts, fill=None):
            if b not in o_t:
                o_t[b] = pO.tile([128, S], BF, tag="ob", name="ob")
            ob = o_t[b]
            qtt, ktt, vt = qt_t[b], kt_t[b], v_t[b]
            for qt_i in qts:
                q0 = qt_i * QT
                nkb = (q0 + QT) // KB
                po = {}
                for h in range(HPC):
                    po[h] = ppo.tile([128, QT], F32, tag=f"po{h}",
                                     name=f"po{h}")

                def pv_quad(kp, pt_h):
                    for i in range(2):
                        kb = 2 * kp + i
                        for h in range(HPC):
                            nc.tensor.matmul(
                                po[h][0:65, :], vt[:, kb, 65 * h:65 * h + 65],
                                pt_h[h][:, i * QT:(i + 1) * QT],
                                start=(kb == 0), stop=(kb == nkb - 1),
                                skip_group_check=True)

                prev = None   # (kp, pt_h) held back one iteration
                for kp in range(nkb // 2):
                    kb0 = 2 * kp
                    # per-head 2-bank tiles: exp ping-pongs between heads
                    ps_h = {h: pps.tile([128, 2 * QT], F32, tag=f"ps{h}",
                                        name=f"ps{h}") for h in range(HPC)}
                    for i in range(2):
                        k0 = (kb0 + i) * KB
                        for h in range(HPC):
                            p0 = h * HD
                            nc.tensor.matmul(
                                ps_h[h][:, i * QT:(i + 1) * QT],
                                ktt[p0:p0 + HD, k0:k0 + KB],
                                qtt[p0:p0 + HD, q0:q0 + QT],
                                start=True, stop=True,
                                skip_group_check=True)
                    if prev is not None:
                        pv_quad(*prev)
                    if fill is not None:
                        f = next(fill, None)
                        if f is not None:
                            f()
                    pt_h = {}
                    for h in range(HPC):
                        pt = ppt.tile([128, 2 * QT], BF, tag=f"pt{h}",
                                      name=f"pt{h}")
                        nc.scalar.activation(
                            pt[:], ps_h[h][:], mybir.ActivationFunctionType.Exp,
                            scale=float(HD) ** -0.5)
                        if kp >= 2 * qt_i:
                            half = kp - 2 * qt_i
                            nc.vector.tensor_tensor(pt[:], pt[:], wmask(half),
                                                    MULT)
                        pt_h[h] = pt
                    prev = (kp, pt_h)
                pv_quad(*prev)
                # normalize: O/l ; l in po row HD; r broadcast via PE
                bcps = ppA.tile([128, QT], F32, tag="pA", name="bcps")
                for h in range(HPC):
                    lsb = pnorm.tile([1, QT], F32, tag=f"lsb{h}", name="lsb")
                    nc.vector.tensor_copy(lsb[:], po[h][HD:HD + 1, :])
                    recip32 = pnorm.tile([1, QT], F32, tag=f"recip32{h}",
                                         name="recip32")
                    nc.vector.reciprocal_approx_fast(recip32[:], lsb[:])
                    recipb = pnorm.tile([1, QT], BF, tag=f"recipb{h}",
                                        name="recipb")
                    nc.vector.tensor_copy(recipb[:], recip32[:])
                    nc.tensor.matmul(
                        bcps[h * HD:(h + 1) * HD, :], ones_sb[:, 0:HD],
                        recipb[:], start=True, stop=True,
                        skip_group_check=True)
                bc = pnorm.tile([2 * HD, QT], BF, tag="bc", name="bc")
                nc.vector.tensor_copy(bc[:], bcps[:])
                for h in range(HPC):
                    nc.vector.tensor_tensor(
                        ob[h * HD:(h + 1) * HD, q0:q0 + QT],
                        po[h][0:HD, :], bc[h * HD:(h + 1) * HD, :], MULT)

        # ---------------- phase C: reshard (collective) ----------------
        def phaseC0():
            for j in range(NC):
                nc.sync.dma_start(
                    out=a2a_in[0][j, :, :],
                    in_=o_t[0][:, j * RH:(j + 1) * RH])
            nc.gpsimd.collective_compute(
                "AllToAll", mybir.AluOpType.bypass,
                replica_groups=[list(range(NC))],
                ins=[a2a_in[0][:].opt()], outs=[a2a_out[0][:].opt()])

        def phaseC1(h):
            base = h * (S // NHALF)
            for j in range(NC):
                nc.sync.dma_start(
                    out=a2a_in[(1, h)][j, :, :],
                    in_=o_t[1][:, base + j * RHH:base + (j + 1) * RHH])
            nc.gpsimd.collective_compute(
                "AllToAll", mybir.AluOpType.bypass,
                replica_groups=[list(range(NC))],
                ins=[a2a_in[(1, h)][:].opt()],
                outs=[a2a_out[(1, h)][:].opt()])

        # ---------------- phase D: out-projection ----------------
        def phaseD0():
            og = pog.tile([128, NC * RH], BF, tag="og", name="og")
            for d in range(NC):
                nc.sync.dma_start(out=og[:, d * RH:(d + 1) * RH],
                                  in_=a2a_out[0][d, :, :])
            osb = posb.tile([128, DCH * RH], BF, tag="osb", name="osb")
            for e in range(DCH):
                pso = ppA.tile([128, RH], F32, tag="pA", name="pso")
                for d in range(NC):
                    nc.tensor.matmul(
                        pso[:], wo_sb[:, d * D + e * 128:d * D + (e + 1) * 128],
                        og[:, d * RH:(d + 1) * RH],
                        start=(d == 0), stop=(d == NC - 1))
                nc.vector.tensor_copy(osb[:, e * RH:(e + 1) * RH], pso[:])
            nc.sync.dma_start(out=out[:, 0:DCH * RH], in_=osb[:])

        def phaseD1(h):
            og = pog.tile([128, NC * RHH], BF, tag="og", name="og1")
            for d in range(NC):
                nc.sync.dma_start(out=og[:, d * RHH:(d + 1) * RHH],
                                  in_=a2a_out[(1, h)][d, :, :])
            osb = posb.tile([128, DCH * RHH], BF, tag="osb", name="osb1")
            for e in range(DCH):
                pso = ppA.tile([128, RHH], F32, tag="pA", name="pso1")
                for d in range(NC):
                    nc.tensor.matmul(
                        pso[:], wo_sb[:, d * D + e * 128:d * D + (e + 1) * 128],
                        og[:, d * RHH:(d + 1) * RHH],
                        start=(d == 0), stop=(d == NC - 1))
                nc.vector.tensor_copy(osb[:, e * RHH:(e + 1) * RHH], pso[:])
            out_v = out[:, DCH * RH:2 * DCH * RH].rearrange(
                "p (e r) -> p e r", e=DCH)[:, :, h * RHH:(h + 1) * RHH]
            nc.sync.dma_start(
                out=out_v,
                in_=osb[:].rearrange("p (e r) -> p e r", e=DCH))

        # ---------------- schedule ----------------
        # b1 Q/K/V projections thread into phaseB(0) bubbles one thunk per
        # kp-pair (PE keeps running while Scalar crunches exps).  C1(1)
        # hides under B(1,[0,1]); C1(0) last, window filled by D0+D1(1).
        phaseA(0)
        thq = iter(a_thunks(1, 'q'))
        phaseB(0, [0, 1], fill=thq)
        drain(thq)
        thk = iter(a_thunks(1, 'k'))
        phaseB(0, [2], fill=thk)
        thv = iter(a_thunks(1, 'v'))
        phaseB(0, [3], fill=chain(thk, thv))
        drain(thk)
        drain(thv)
        phaseC0()
        phaseB(1, [2, 3])
        phaseC1(1)
        phaseB(1, [0, 1])
        phaseC1(0)
        phaseD0()
        phaseD1(1)
        phaseD1(0)

    nc.compile()
    return nc


_NC_CACHE = {}


def _get_nc(S):
    if S not in _NC_CACHE:
        _NC_CACHE[S] = build_nc(S)
    return _NC_CACHE[S]


def make_in_maps(x, wq, wk, wv, wo):
    b, S, d = x.shape
    NST = S // QT

    # xP[p, b*NST+st, d*QT+c] = x[b, st*QT+c, d*128+p]
    xP = np.ascontiguousarray(
        x.reshape(B, NST, QT, DCH, 128).transpose(4, 0, 1, 3, 2)
        .reshape(128, B * NST, DCH * QT)).astype(BF16)

    # woP[p, d*D+e] = wo[e, d*128+p]
    woP = np.ascontiguousarray(
        wo.T.reshape(DCH, 128, D).transpose(1, 0, 2).reshape(128, DCH * D)
    ).astype(BF16)

    # RoPE tables (transposed, both 64-dim head copies stacked)
    inv = (1.0 / ROPE_BASE ** (np.arange(0, HD, 2, dtype=np.float64) / HD))
    t = np.arange(S, dtype=np.float64)
    fr = np.outer(t, inv)                      # [S, 32]
    emb = np.concatenate([fr, fr], axis=1)     # [S, 64]
    cos_t = np.cos(emb).T                      # [64, S]
    sin_t = np.sin(emb).T
    sgn = np.where(np.arange(HD) < HD // 2, -1.0, 1.0)[:, None]
    cosT = np.concatenate([cos_t, cos_t], axis=0)          # [128, S]
    sinT = np.concatenate([sin_t * sgn, sin_t * sgn], axis=0)

    pp = np.arange(128)[:, None]
    qn = np.arange(QT)[None, :]
    # wide masks: [mask_0|mask_1] and [mask_2|mask_3]
    masks = [(qn >= j * KB + pp) for j in range(4)]
    wm01 = np.concatenate([masks[0], masks[1]], axis=1)
    wm23 = np.concatenate([masks[2], masks[3]], axis=1)

    csm = np.concatenate([cosT, sinT, wm01, wm23], axis=1).astype(BF16)
    assert csm.shape[1] == CSM_COLS

    def wpack(w, sl):
        return w[sl, :].T.reshape(DCH, 128, DPC).transpose(1, 0, 2).reshape(
            128, DCH * DPC)

    in_maps = []
    for c in range(NC):
        sl = slice(c * DPC, (c + 1) * DPC)
        wqkv = np.concatenate(
            [wpack(wq, sl), wpack(wk, sl), wpack(wv, sl)], axis=1).astype(BF16)
        in_maps.append({
            "wqkv": np.ascontiguousarray(wqkv),
            "csm": np.ascontiguousarray(csm),
            "woP": woP,
            "xP": xP,
        })
    return in_maps


def assemble(outs, S):
    """outs[c] [128, B*DCH*RH] -> full (B, S, D) f32.

    Batch 0: contiguous RH rows per core. Batch 1: two halves of RHH.
    """
    RH = S // NC
    RHH = RH // 2
    full = np.empty((B, S, D), dtype=np.float32)
    for c in range(NC):
        o = np.asarray(outs[c]).astype(np.float32).reshape(128, B, DCH, RH)
        blk = o.transpose(1, 3, 2, 0).reshape(B, RH, D)
        full[0, c * RH:(c + 1) * RH, :] = blk[0]
        for h in range(2):
            r0 = h * (S // 2) + c * RHH
            full[1, r0:r0 + RHH, :] = blk[1, h * RHH:(h + 1) * RHH]
    return full


def run(x, wq, wk, wv, wo, trace=False):
    b, S, d = x.shape
    nc = _get_nc(S)
    in_maps = make_in_maps(x, wq, wk, wv, wo)
    res = run_bass_kernel_spmd(nc, in_maps, core_ids=list(range(NC)),
                               trace=trace)
    full = assemble([res.results[c]["out"] for c in range(NC)], S)
    return full, res


def kernel(x, wq, wk, wv, wo):
    full, _ = run(np.asarray(x), np.asarray(wq), np.asarray(wk),
                  np.asarray(wv), np.asarray(wo))
    return full



# revision 38
# speedup vs baseline: 1.0382x; 1.0024x over previous
"""Multi-head attention (RoPE + causal SDPA) on 8 Trainium2 NeuronCores.

Tensor-parallel over heads (2 heads/core) for QKV+attention; per-(batch,
half) AllToAll reshards head-split -> row-split; each core out-projects
its row slice with the full wo.

v3 notes:
  - Packed DRAM layouts; big-descriptor DMAs only.
  - V computed in natural (kpos, hd) layout (x-chunk stationary).
  - Two heads' K=64 score matmuls issued back-to-back -> concurrent PE
    row-tiles (0,0)/(64,0) (verified in trace: 2nd of pair ~3ns).
  - Scores for a kb-pair land in one 2-bank PSUM tile [128,1024]; one
    exp ACTIVATE covers both blocks. Causal masks as ONE wide [128,1024]
    tensor_tensor per (qtile, head, diag-pair), split DVE/GpSimd.
  - Elementwise rebalanced: GpSimd only gets full-128-partition
    SBUF-only multiplies (it is ~2.6x slower than DVE and terrible on
    <128-partition ops); DVE keeps PSUM readers + small copies.
  - phaseA(1) interleaved into phaseB(0) at q-tile granularity so PE has
    dense work while Scalar crunches exps; collectives start early.
  - Output stored bf16 (host casts to f32).
"""
import sys, os
if '/opt/trn_rl_repo' not in sys.path:
    sys.path.insert(0, '/opt/trn_rl_repo')
os.environ.setdefault('MYCRO_LOCAL_CACHE', '1')

from contextlib import ExitStack
from itertools import chain

import numpy as np
import ml_dtypes

import concourse.bass as bass
import concourse.tile as tile
from concourse import bacc, mybir
from concourse.bass_utils import run_bass_kernel_spmd

BF16 = ml_dtypes.bfloat16
NC = 8           # cores
B = 2            # batch
D = 1024         # model dim
H = 16           # heads
HD = 64          # head dim
HPC = H // NC    # heads per core = 2
DPC = HPC * HD   # head dims per core = 128
ROPE_BASE = 10000.0
QT = 512         # q tile width
KB = 128         # k block (partition axis of score matmuls)
DCH = D // 128   # contraction chunks (= 8)

F32 = mybir.dt.float32
BF = mybir.dt.bfloat16
FP8 = mybir.dt.float8e4
FP8NP = ml_dtypes.float8_e4m3fn
WSCALE = 64.0      # wq/wk/wv scaled by 64 so fp8e4 values stay normal
MULT = mybir.AluOpType.mult
ADD = mybir.AluOpType.add

# csm (cos/sin/masks) column offsets
COS0, SIN0, MSK0 = 0, 2048, 4096
CSM_COLS = MSK0 + 2 * 2 * QT     # cos|sin|wmask01|wmask23 = 6144


def build_nc(S):
    RH = S // NC           # rows per core per batch = 256
    NST = S // QT          # 4 seq tiles per batch
    NVT = S // KB          # 16 V k-blocks per batch
    NHALF = 2
    RHH = RH // NHALF      # 128

    nc = bacc.Bacc(num_devices=NC)

    wqkv = nc.declare_dram_parameter("wqkv", [128, 3 * D], BF, isOutput=False)
    csm = nc.declare_dram_parameter("csm", [128, CSM_COLS], BF, isOutput=False)
    woP = nc.declare_dram_parameter("woP", [128, DCH * D], BF, isOutput=False)
    xP = nc.declare_dram_parameter("xP", [128, B * NST, DCH * QT], BF,
                                   isOutput=False)
    out = nc.declare_dram_parameter("out", [128, B * DCH * RH], BF,
                                    isOutput=True)

    a2a_in = {0: nc.dram_tensor("a2a_in0", [NC, 128, RH], BF)}
    a2a_out = {0: nc.dram_tensor("a2a_out0", [NC, 128, RH], BF)}
    for h in range(NHALF):
        a2a_in[(1, h)] = nc.dram_tensor(f"a2a_in1_{h}", [NC, 128, RHH], BF)
        a2a_out[(1, h)] = nc.dram_tensor(f"a2a_out1_{h}", [NC, 128, RHH], BF)
    dsync_in = nc.dram_tensor("dsync_in", [NC, 1, 16], BF)
    dsync_out = nc.dram_tensor("dsync_out", [NC, 1, 16], BF)

    ctx = ExitStack()
    with ctx:
        tc = ctx.enter_context(tile.TileContext(nc))

        consts = ctx.enter_context(tc.tile_pool(name="consts", bufs=1))
        xpool = ctx.enter_context(tc.tile_pool(name="x", bufs=2 * NST))
        pQt = ctx.enter_context(tc.tile_pool(name="qt", bufs=2))
        pKt = ctx.enter_context(tc.tile_pool(name="kt", bufs=2))
        pV = ctx.enter_context(tc.tile_pool(name="v", bufs=2))
        pO = ctx.enter_context(tc.tile_pool(name="oall", bufs=2))
        ptmp = ctx.enter_context(tc.tile_pool(name="tmp", bufs=4))
        ppt = ctx.enter_context(tc.tile_pool(name="pt", bufs=2))
        pnorm = ctx.enter_context(tc.tile_pool(name="norm", bufs=3))
        pog = ctx.enter_context(tc.tile_pool(name="og", bufs=2))
        posb = ctx.enter_context(tc.tile_pool(name="osb", bufs=2))

        # PSUM: 8 banks = ppA(2) + ps(4: both heads x kb-pair) + po0/po1(2)
        ppA = ctx.enter_context(tc.tile_pool(name="ppA", bufs=2, space="PSUM"))
        pps = ctx.enter_context(tc.tile_pool(name="pps", bufs=1, space="PSUM"))
        ppo = ctx.enter_context(tc.tile_pool(name="ppo", bufs=1, space="PSUM"))

        # launch-skew absorber: cores rendezvous here while input DMAs
        # stream in (saves the first real collective from eating the skew)
        nc.gpsimd.collective_compute(
            "AllToAll", mybir.AluOpType.bypass,
            replica_groups=[list(range(NC))],
            ins=[dsync_in[:].opt()], outs=[dsync_out[:].opt()])

        # ---- constants ----
        # wqkv split q|k|v so the first LDWEIGHTS waits only on the q part
        wqkv_sb = consts.tile([128, 3 * D], BF, tag="wqkv")
        nc.sync.dma_start(out=wqkv_sb[:, 0:D], in_=wqkv[:, 0:D])

        def w_sl(which, d):
            base = {'q': 0, 'k': D, 'v': 2 * D}[which]
            return wqkv_sb[:, base + d * 128:base + (d + 1) * 128]

        def w_pair(which, d):   # chunks d,d+1 as [128, 2, 128] for DoubleRow
            base = {'q': 0, 'k': D, 'v': 2 * D}[which]
            return wqkv_sb[:, base + d * 128:base + (d + 2) * 128].rearrange(
                "p (j m) -> p j m", j=2)

        # x seq-blocks: batch0 st0, then csm, then rest
        xts = {}

        def load_x(b, st, chunks=1, eng=None):
            t = xpool.tile([128, DCH * QT], BF, tag="x", name="xt")
            cw = (DCH // chunks) * QT
            for ci in range(chunks):
                (eng or nc.sync).dma_start(out=t[:, ci * cw:(ci + 1) * cw],
                                           in_=xP[:, b * NST + st,
                                                  ci * cw:(ci + 1) * cw])
            xts[(b, st)] = t

        load_x(0, 0, chunks=8)
        nc.sync.dma_start(out=wqkv_sb[:, D:3 * D], in_=wqkv[:, D:3 * D])
        load_x(0, 1)
        csm_sb = consts.tile([128, CSM_COLS], BF, tag="csm")
        nc.sync.dma_start(out=csm_sb[:, 0:MSK0], in_=csm[:, 0:MSK0])
        cos_sb = csm_sb[:, COS0:COS0 + S]
        sin_sb = csm_sb[:, SIN0:SIN0 + S]

        def wmask(half):   # [128, 1024] wide causal mask for diag pair
            return csm_sb[:, MSK0 + half * 2 * QT:MSK0 + (half + 1) * 2 * QT]

        load_x(0, 2)
        nc.sync.dma_start(out=csm_sb[:, MSK0:CSM_COLS],
                          in_=csm[:, MSK0:CSM_COLS])
        load_x(0, 3)
        ones_sb = consts.tile([1, HD], BF, tag="ones")
        nc.gpsimd.memset(ones_sb[:], 1.0)
        for st in range(NST):
            load_x(1, st)
        wo_sb = consts.tile([128, DCH * D], BF, tag="wo")
        nc.sync.dma_start(out=wo_sb[:], in_=woP[:, :])

        qt_t, kt_t, v_t, o_t = {}, {}, {}, {}
        for _b in range(B):
            _vt = pV.tile([128, NVT, 130], BF, tag="vt", name="vt")
            nc.gpsimd.memset(_vt[:, :, 64:65], 1.0)
            nc.gpsimd.memset(_vt[:, :, 129:130], 1.0)
            v_t[_b] = _vt

        # ---------------- phase A: st-major Q/K/V interleave ----------------
        # Emitted as small thunks (~4 matmuls each) so phaseB can pop one
        # per kp-pair, filling the PE bubble while Scalar crunches exps.
        def proj_rope_thunks(b, which, st):
            xt = xts[(b, st)]
            box = {}

            def t1():
                ps = ppA.tile([128, QT], F32, tag="pA", name="psproj")
                box['ps'] = ps
                for d in range(4):
                    nc.tensor.matmul(
                        ps[:], w_sl(which, d), xt[:, d * QT:(d + 1) * QT],
                        start=(d == 0), stop=False)

            def t2():
                ps = box['ps']
                dest = qt_t[b] if which == 'q' else kt_t[b]
                for d in range(4, DCH):
                    nc.tensor.matmul(
                        ps[:], w_sl(which, d), xt[:, d * QT:(d + 1) * QT],
                        start=False, stop=(d == DCH - 1))
                c0 = st * QT
                raw = ptmp.tile([128, QT], BF, tag="raw", name="raw")
                if b == 0:
                    nc.scalar.activation(raw[:], ps[:],
                                         mybir.ActivationFunctionType.Copy)
                else:
                    nc.vector.tensor_copy(raw[:], ps[:])
                tcos = ptmp.tile([128, QT], BF, tag="tcos", name="tcos")
                nc.vector.tensor_tensor(
                    tcos[:], raw[:], cos_sb[:, c0:c0 + QT], MULT)
                trot = ptmp.tile([128, QT], BF, tag="trot", name="trot")
                for g in range(4):
                    o0 = g * 32
                    i0 = o0 + 32 if g % 2 == 0 else o0 - 32
                    nc.vector.tensor_copy(trot[o0:o0 + 32, :],
                                          raw[i0:i0 + 32, :])
                nc.vector.tensor_tensor(
                    trot[:], trot[:], sin_sb[:, c0:c0 + QT], MULT)
                nc.vector.tensor_tensor(
                    dest[:, c0:c0 + QT], tcos[:], trot[:], ADD)

            return [t1, t2]

        def proj_v_thunks(b, st):
            xt = xts[(b, st)]
            box = {}

            def mk(i0):
                def t():
                    if 'ps' not in box:
                        box['ps'] = ppA.tile([128, 4 * 128], F32, tag="pA",
                                             name="psv")
                    psv = box['ps']
                    for i in (i0, i0 + 1):
                        kcols = i * 128
                        for d in range(DCH):
                            nc.tensor.matmul(
                                psv[:, kcols:kcols + 128],
                                xt[:, d * QT + kcols:d * QT + kcols + 128],
                                w_sl('v', d),
                                start=(d == 0), stop=(d == DCH - 1),
                                skip_group_check=True)
                    if i0 == 2:
                        vt = v_t[b]
                        pv3 = psv[:].rearrange("p (k c) -> p k c", c=128)
                        vt3 = vt[:, st * 4:(st + 1) * 4, :]
                        nc.vector.tensor_copy(vt3[:, :, 0:64], pv3[:, :, 0:64])
                        nc.vector.tensor_copy(vt3[:, :, 65:129],
                                              pv3[:, :, 64:128])
                return t

            return [mk(0), mk(2)]

        def a_thunks(b, parts):
            if 'q' in parts and b not in qt_t:
                qt_t[b] = pQt.tile([128, S], BF, tag="qt", name="qtt")
            if 'k' in parts and b not in kt_t:
                kt_t[b] = pKt.tile([128, S], BF, tag="kt", name="ktt")
            th = []
            for st in range(NST):
                for p in parts:
                    if p == 'v':
                        th += proj_v_thunks(b, st)
                    else:
                        th += proj_rope_thunks(b, p, st)
            return th

        def phaseA(b, parts='qkv'):
            for t in a_thunks(b, parts):
                t()

        def drain(it):
            for t in it:
                t()

        # ---------------- phase B: attention ----------------
        def phaseB(b, qts, fill=None):
            if b not in o_t:
                o_t[b] = pO.tile([128, S], BF, tag="ob", name="ob")
            ob = o_t[b]
            qtt, ktt, vt = qt_t[b], kt_t[b], v_t[b]
            for qt_i in qts:
                q0 = qt_i * QT
                nkb = (q0 + QT) // KB
                po = {}
                for h in range(HPC):
                    po[h] = ppo.tile([128, QT], F32, tag=f"po{h}",
                                     name=f"po{h}")

                def pv_quad(kp, pt_h):
                    for i in range(2):
                        kb = 2 * kp + i
                        for h in range(HPC):
                            nc.tensor.matmul(
                                po[h][0:65, :], vt[:, kb, 65 * h:65 * h + 65],
                                pt_h[h][:, i * QT:(i + 1) * QT],
                                start=(kb == 0), stop=(kb == nkb - 1),
                                skip_group_check=True)

                prev = None   # (kp, pt_h) held back one iteration
                for kp in range(nkb // 2):
                    kb0 = 2 * kp
                    # per-head 2-bank tiles: exp ping-pongs between heads
                    ps_h = {h: pps.tile([128, 2 * QT], F32, tag=f"ps{h}",
                                        name=f"ps{h}") for h in range(HPC)}
                    for i in range(2):
                        k0 = (kb0 + i) * KB
                        for h in range(HPC):
                            p0 = h * HD
                            nc.tensor.matmul(
                                ps_h[h][:, i * QT:(i + 1) * QT],
                                ktt[p0:p0 + HD, k0:k0 + KB],
                                qtt[p0:p0 + HD, q0:q0 + QT],
                                start=True, stop=True,
                                skip_group_check=True)
                    if prev is not None:
                        pv_quad(*prev)
                    if fill is not None:
                        f = next(fill, None)
                        if f is not None:
                            f()
                    pt_h = {}
                    for h in range(HPC):
                        pt = ppt.tile([128, 2 * QT], BF, tag=f"pt{h}",
                                      name=f"pt{h}")
                        nc.scalar.activation(
                            pt[:], ps_h[h][:], mybir.ActivationFunctionType.Exp,
                            scale=float(HD) ** -0.5)
                        if kp >= 2 * qt_i:
                            half = kp - 2 * qt_i
                            nc.vector.tensor_tensor(pt[:], pt[:], wmask(half),
                                                    MULT)
                        pt_h[h] = pt
                    prev = (kp, pt_h)
                pv_quad(*prev)
                # normalize: O/l ; l in po row HD; r broadcast via PE
                bcps = ppA.tile([128, QT], F32, tag="pA", name="bcps")
                for h in range(HPC):
                    lsb = pnorm.tile([1, QT], F32, tag=f"lsb{h}", name="lsb")
                    nc.vector.tensor_copy(lsb[:], po[h][HD:HD + 1, :])
                    recip32 = pnorm.tile([1, QT], F32, tag=f"recip32{h}",
                                         name="recip32")
                    nc.vector.reciprocal_approx_fast(recip32[:], lsb[:])
                    recipb = pnorm.tile([1, QT], BF, tag=f"recipb{h}",
                                        name="recipb")
                    nc.vector.tensor_copy(recipb[:], recip32[:])
                    nc.tensor.matmul(
                        bcps[h * HD:(h + 1) * HD, :], ones_sb[:, 0:HD],
                        recipb[:], start=True, stop=True,
                        skip_group_check=True)
                bc = pnorm.tile([2 * HD, QT], BF, tag="bc", name="bc")
                nc.vector.tensor_copy(bc[:], bcps[:])
                for h in range(HPC):
                    nc.vector.tensor_tensor(
                        ob[h * HD:(h + 1) * HD, q0:q0 + QT],
                        po[h][0:HD, :], bc[h * HD:(h + 1) * HD, :], MULT)

        # ---------------- phase C: reshard (collective) ----------------
        def phaseC0():
            for j in range(NC):
                nc.sync.dma_start(
                    out=a2a_in[0][j, :, :],
                    in_=o_t[0][:, j * RH:(j + 1) * RH])
            nc.gpsimd.collective_compute(
                "AllToAll", mybir.AluOpType.bypass,
                replica_groups=[list(range(NC))],
                ins=[a2a_in[0][:].opt()], outs=[a2a_out[0][:].opt()])

        def phaseC1(h):
            base = h * (S // NHALF)
            for j in range(NC):
                nc.sync.dma_start(
                    out=a2a_in[(1, h)][j, :, :],
                    in_=o_t[1][:, base + j * RHH:base + (j + 1) * RHH])
            nc.gpsimd.collective_compute(
                "AllToAll", mybir.AluOpType.bypass,
                replica_groups=[list(range(NC))],
                ins=[a2a_in[(1, h)][:].opt()],
                outs=[a2a_out[(1, h)][:].opt()])

        # ---------------- phase D: out-projection ----------------
        def phaseD0():
            og = pog.tile([128, NC * RH], BF, tag="og", name="og")
            for d in range(NC):
                nc.sync.dma_start(out=og[:, d * RH:(d + 1) * RH],
                                  in_=a2a_out[0][d, :, :])
            osb = posb.tile([128, DCH * RH], BF, tag="osb", name="osb")
            for e in range(DCH):
                pso = ppA.tile([128, RH], F32, tag="pA", name="pso")
                for d in range(NC):
                    nc.tensor.matmul(
                        pso[:], wo_sb[:, d * D + e * 128:d * D + (e + 1) * 128],
                        og[:, d * RH:(d + 1) * RH],
                        start=(d == 0), stop=(d == NC - 1))
                nc.vector.tensor_copy(osb[:, e * RH:(e + 1) * RH], pso[:])
            nc.sync.dma_start(out=out[:, 0:DCH * RH], in_=osb[:])

        def phaseD1(h):
            og = pog.tile([128, NC * RHH], BF, tag="og", name="og1")
            for d in range(NC):
                nc.sync.dma_start(out=og[:, d * RHH:(d + 1) * RHH],
                                  in_=a2a_out[(1, h)][d, :, :])
            osb = posb.tile([128, DCH * RHH], BF, tag="osb", name="osb1")
            for e in range(DCH):
                pso = ppA.tile([128, RHH], F32, tag="pA", name="pso1")
                for d in range(NC):
                    nc.tensor.matmul(
                        pso[:], wo_sb[:, d * D + e * 128:d * D + (e + 1) * 128],
                        og[:, d * RHH:(d + 1) * RHH],
                        start=(d == 0), stop=(d == NC - 1))
                nc.vector.tensor_copy(osb[:, e * RHH:(e + 1) * RHH], pso[:])
            out_v = out[:, DCH * RH:2 * DCH * RH].rearrange(
                "p (e r) -> p e r", e=DCH)[:, :, h * RHH:(h + 1) * RHH]
            nc.sync.dma_start(
                out=out_v,
                in_=osb[:].rearrange("p (e r) -> p e r", e=DCH))

        # ---------------- schedule ----------------
        # b1 Q/K/V projections thread into phaseB(0) bubbles one thunk per
        # kp-pair (PE keeps running while Scalar crunches exps).  C1(1)
        # hides under B(1,[0,1]); C1(0) last, window filled by D0+D1(1).
        phaseA(0)
        thq = iter(a_thunks(1, 'q'))
        phaseB(0, [0, 1], fill=thq)
        drain(thq)
        thk = iter(a_thunks(1, 'k'))
        phaseB(0, [2], fill=thk)
        thv = iter(a_thunks(1, 'v'))
        phaseB(0, [3], fill=chain(thk, thv))
        drain(thk)
        drain(thv)
        phaseC0()
        phaseB(1, [2, 3])
        phaseC1(1)
        phaseB(1, [0, 1])
        phaseC1(0)
        phaseD0()
        phaseD1(1)
        phaseD1(0)

    nc.compile()
    return nc


_NC_CACHE = {}


def _get_nc(S):
    if S not in _NC_CACHE:
        _NC_CACHE[S] = build_nc(S)
    return _NC_CACHE[S]


def make_in_maps(x, wq, wk, wv, wo):
    b, S, d = x.shape
    NST = S // QT

    # xP[p, b*NST+st, d*QT+c] = x[b, st*QT+c, d*128+p]
    xP = np.ascontiguousarray(
        x.reshape(B, NST, QT, DCH, 128).transpose(4, 0, 1, 3, 2)
        .reshape(128, B * NST, DCH * QT)).astype(BF16)

    # woP[p, d*D+e] = wo[e, d*128+p]
    woP = np.ascontiguousarray(
        wo.T.reshape(DCH, 128, D).transpose(1, 0, 2).reshape(128, DCH * D)
    ).astype(BF16)

    # RoPE tables (transposed, both 64-dim head copies stacked)
    inv = (1.0 / ROPE_BASE ** (np.arange(0, HD, 2, dtype=np.float64) / HD))
    t = np.arange(S, dtype=np.float64)
    fr = np.outer(t, inv)                      # [S, 32]
    emb = np.concatenate([fr, fr], axis=1)     # [S, 64]
    cos_t = np.cos(emb).T                      # [64, S]
    sin_t = np.sin(emb).T
    sgn = np.where(np.arange(HD) < HD // 2, -1.0, 1.0)[:, None]
    cosT = np.concatenate([cos_t, cos_t], axis=0)          # [128, S]
    sinT = np.concatenate([sin_t * sgn, sin_t * sgn], axis=0)

    pp = np.arange(128)[:, None]
    qn = np.arange(QT)[None, :]
    # wide masks: [mask_0|mask_1] and [mask_2|mask_3]
    masks = [(qn >= j * KB + pp) for j in range(4)]
    wm01 = np.concatenate([masks[0], masks[1]], axis=1)
    wm23 = np.concatenate([masks[2], masks[3]], axis=1)

    csm = np.concatenate([cosT, sinT, wm01, wm23], axis=1).astype(BF16)
    assert csm.shape[1] == CSM_COLS

    def wpack(w, sl):
        return w[sl, :].T.reshape(DCH, 128, DPC).transpose(1, 0, 2).reshape(
            128, DCH * DPC)

    in_maps = []
    for c in range(NC):
        sl = slice(c * DPC, (c + 1) * DPC)
        wqkv = np.concatenate(
            [wpack(wq, sl), wpack(wk, sl), wpack(wv, sl)], axis=1).astype(BF16)
        in_maps.append({
            "wqkv": np.ascontiguousarray(wqkv),
            "csm": np.ascontiguousarray(csm),
            "woP": woP,
            "xP": xP,
        })
    return in_maps


def assemble(outs, S):
    """outs[c] [128, B*DCH*RH] -> full (B, S, D) f32.

    Batch 0: contiguous RH rows per core. Batch 1: two halves of RHH.
    """
    RH = S // NC
    RHH = RH // 2
    full = np.empty((B, S, D), dtype=np.float32)
    for c in range(NC):
        o = np.asarray(outs[c]).astype(np.float32).reshape(128, B, DCH, RH)
        blk = o.transpose(1, 3, 2, 0).reshape(B, RH, D)
        full[0, c * RH:(c + 1) * RH, :] = blk[0]
        for h in range(2):
            r0 = h * (S // 2) + c * RHH
            full[1, r0:r0 + RHH, :] = blk[1, h * RHH:(h + 1) * RHH]
    return full


def run(x, wq, wk, wv, wo, trace=False):
    b, S, d = x.shape
    nc = _get_nc(S)
    in_maps = make_in_maps(x, wq, wk, wv, wo)
    res = run_bass_kernel_spmd(nc, in_maps, core_ids=list(range(NC)),
                               trace=trace)
    full = assemble([res.results[c]["out"] for c in range(NC)], S)
    return full, res


def kernel(x, wq, wk, wv, wo):
    full, _ = run(np.asarray(x), np.asarray(wq), np.asarray(wk),
                  np.asarray(wv), np.asarray(wo))
    return full

